# revision 1
# baseline (speedup 1.0000x reference)
"""Trainium2 Bass kernel for nn_CompressSensory (embedding_lookup):
out = twohot_table[argmax(x, axis=1)] for x [1048576, 45] f32.

Strategy: pure data parallel over 8 NeuronCores (131072 rows each). The 45
features decompose by the two-hot table's triangular structure: table row
idx = Tri(g-1)+r has set bits at columns 9-g and 9-r. Per input row:
  - group maxes M_g over the 9 contiguous triangular spans (DVE grouped
    free-axis reduces)
  - offset maxes acc_r = max_{g>r} x[Tri(g-1)+r] via a shifted
    tensor-tensor max chain (GpSimd, overlapped with DVE)
  - row max m; then out col 0 = (M_9==m), col c = (max(M_{9-c},acc_{9-c})==m)
    for c=1..8, col 9 = (acc_0==m) — equality one-hots written straight
    into the output tile with reversed APs (a* != b* makes OR==ADD valid).
Exact fp32 equality keeps argmax semantics except on exact ties (one row
in this dataset); any tie yields a row-sum != 2, fixed up exactly on host.
"""

import os

import numpy as np

# Whole-tile dep granularity keeps per-instruction sync-wait counts low
# (walrus rejects DMA pseudo-instructions with >1 sync wait).
os.environ.setdefault("BY_DEFAULT_DISABLE_SUBTILE_DEPS", "1")

import concourse.bass as bass
import concourse.bacc as bacc
import concourse.mybir as mybir
from concourse.tile import TileContext
from concourse.bass_utils import run_bass_kernel_spmd

F32 = mybir.dt.float32
N_CORES = 8
ROWS_TOTAL = 1048576
X_DIM = 45
OUT_DIM = 10
ROWS = ROWS_TOTAL // N_CORES  # 131072 per core
P = 128                       # SBUF partitions
R = int(os.environ.get("K_R", "64"))          # rows per partition per tile
NTILES = ROWS // (P * R)
# Note: this neuronxcc/walrus build rejects TensorTensor on Pool
# ("Instruction engine check failed"), so the chain stays on DVE.
CHAIN_ENG = os.environ.get("K_CHAIN_ENG", "vector")
XT_BUFS = int(os.environ.get("K_XT_BUFS", "4"))
POOL_BUFS = int(os.environ.get("K_POOL_BUFS", "3"))
# Timing aid: repeat the whole tile sweep in-device so exec time dominates
# dispatch overhead. Output is idempotent across repeats. Default 1.
REPEAT = int(os.environ.get("K_REPEAT", "1"))
TRI = [g * (g - 1) // 2 for g in range(1, 11)]

_CACHE = {}


def _build_nc():
    # Bacc (not bare Bass): finalize() runs generate_event_semaphores, which
    # splits multi-wait DMAs into event-semaphore + 1-wait DMA pairs.
    nc = bacc.Bacc()
    x_d = nc.declare_dram_parameter("x", [ROWS, X_DIM], F32, isOutput=False)
    o_d = nc.declare_dram_parameter("o", [ROWS, OUT_DIM], F32, isOutput=True)

    x_v = x_d.rearrange("(n p r) d -> n p (r d)", p=P, r=R)
    o_v = o_d.rearrange("(n p r) e -> n p (r e)", p=P, r=R)

    with TileContext(nc) as tc:
        with tc.tile_pool(name="pool", bufs=POOL_BUFS) as pool:
            for n in [t for _ in range(REPEAT) for t in range(NTILES)]:
                xt = pool.tile([P, R * X_DIM], F32, tag="xt", bufs=XT_BUFS)
                nc.sync.dma_start(xt[:], x_v[n])
                x3 = xt.rearrange("p (r d) -> p r d", d=X_DIM)

                eng = nc.gpsimd if CHAIN_ENG == "gpsimd" else nc.vector

                # group maxes M_g (slot g-1), g=1..9
                Mst = pool.tile([P, R * 9], F32, tag="Mst")
                M3 = Mst.rearrange("p (r g) -> p r g", g=9)
                for g in range(1, 10):
                    nc.vector.tensor_reduce(
                        M3[:, :, g - 1], x3[:, :, TRI[g - 1]:TRI[g]],
                        axis=mybir.AxisListType.X, op=mybir.AluOpType.max,
                    )

                # offset-max chain, init folded: acc[0:8]=max(grp9[0:8],grp8)
                acc = pool.tile([P, R * 9], F32, tag="acc")
                a3 = acc.rearrange("p (r g) -> p r g", g=9)
                eng.tensor_tensor(
                    a3[:, :, 0:8], x3[:, :, TRI[8]:TRI[8] + 8],
                    x3[:, :, TRI[7]:TRI[8]], mybir.AluOpType.max,
                )
                eng.tensor_copy(a3[:, :, 8:9], x3[:, :, TRI[8] + 8:TRI[9]])
                for g in range(7, 0, -1):
                    eng.tensor_tensor(
                        a3[:, :, 0:g], a3[:, :, 0:g],
                        x3[:, :, TRI[g - 1]:TRI[g]], mybir.AluOpType.max,
                    )

                mrow = pool.tile([P, R], F32, tag="mrow")
                nc.vector.tensor_reduce(
                    mrow[:], a3, axis=mybir.AxisListType.X,
                    op=mybir.AluOpType.max,
                )
                m_b9 = mrow.unsqueeze(2).broadcast_to([P, R, 9])
                m_b1 = mrow.unsqueeze(2).broadcast_to([P, R, 1])

                ot = pool.tile([P, R * OUT_DIM], F32, tag="ot")
                o3 = ot.rearrange("p (r e) -> p r e", e=10)

                # V-merge: slot k (k=1..8) feeds out col 9-k, needs group k
                # (M3 slot k-1): acc[1:9] = max(acc[1:9], M3[0:8]) in place
                nc.vector.tensor_tensor(
                    a3[:, :, 1:9], a3[:, :, 1:9], M3[:, :, 0:8],
                    mybir.AluOpType.max,
                )
                # cols 9..1 <- eq(acc[0:9], m) (reversed out AP)
                nc.vector.tensor_tensor(
                    o3[:, :, 1:10][:, :, ::-1], a3, m_b9,
                    mybir.AluOpType.is_equal,
                )
                # col 0 <- eq(M_9, m)
                nc.vector.tensor_tensor(
                    o3[:, :, 0:1], M3[:, :, 8:9], m_b1,
                    mybir.AluOpType.is_equal,
                )

                nc.sync.dma_start(o_v[n], ot[:])
    return nc


def _get_nc():
    if "nc" not in _CACHE:
        nc = _build_nc()
        if not nc.is_finalized():
            nc.finalize()  # Bacc: alloc_regs + generate_event_semaphores
        _CACHE["nc"] = nc
    return _CACHE["nc"]


def run_on_hw(x, trace=False, **kw):
    """Run the SPMD kernel on the 8 cores; returns (out, results)."""
    nc = _get_nc()
    shards = x.reshape(N_CORES, ROWS, X_DIM)
    in_maps = [{"x": np.ascontiguousarray(shards[c])} for c in range(N_CORES)]
    res = run_bass_kernel_spmd(nc, in_maps, list(range(N_CORES)), trace=trace, **kw)
    out = np.concatenate([np.asarray(r["o"]) for r in res.results], axis=0)
    return out, res


def kernel(x, twohot_table):
    x = np.asarray(x, dtype=np.float32)
    table = np.asarray(twohot_table, dtype=np.float32)
    assert x.shape == (ROWS_TOTAL, X_DIM), x.shape

    out, _ = run_on_hw(x)

    # Exact-tie fixup: equality-based argmax sets >2 bits on tied rows
    # (row-sum != 2). Recompute those rows exactly on host.
    bad = out.sum(axis=1) != 2.0
    if bad.any():
        out[bad] = table[x[bad].argmax(axis=1)]
    return out



# revision 14
# speedup vs baseline: 51.9198x; 51.9198x over previous
"""Trainium2 Bass kernel for nn_CompressSensory (embedding_lookup):
out = twohot_table[argmax(x, axis=1)] for x [1048576, 45] f32.

Strategy: pure data parallel over 8 NeuronCores (131072 rows each), in a
TRANSPOSED feature-major layout. The host ships x as bf16 [45, 131072] per
core; each feature row is one [128, 1024] SBUF tile (a single contiguous
262KB DMA, 45 total). All per-row work is then pure elementwise [128,1024]
tile ops on the DVE, where bf16 tensor_tensor runs in 2x perf mode:
  group maxes M_g  = TT-max fold over the g features of group g     (36 TT)
  offset maxes A_r = TT-max fold over features Tri(g-1)+r, g>r      (36 TT)
  row max m        = TT-max fold over the 9 group maxes              (8 TT)
  V-merge          = A_r <- max(A_r, M_r) for r=1..8                 (8 TT)
  one-hots         = eq(M_9,m) -> col 0; eq(A_{9-c},m) -> col c      (10 TT)
The eq results go to contiguous tiles (2x mode); the ACT engine interleaves
them into the [128, 1024*10] output tile in parallel with DVE; one 2.6MB
DMA stores o [131072, 10] bf16.

Correctness: bf16 rounding is monotone, so vs the f32 reference the argmax
can only TIE (never silently flip); any tie sets >=3 bits -> row-sum != 2
-> those rows (~1.2%) are recomputed exactly on host from f32 x.
"""

import os

import numpy as np

# Whole-tile dep granularity keeps per-instruction sync-wait counts low
# (walrus rejects DMA pseudo-instructions with >1 sync wait).
os.environ.setdefault("BY_DEFAULT_DISABLE_SUBTILE_DEPS", "1")

import concourse.bass as bass
import concourse.bacc as bacc
import concourse.mybir as mybir
from concourse.tile import TileContext
from concourse.bass_utils import run_bass_kernel_spmd

F32 = mybir.dt.float32
N_CORES = 8
ROWS_TOTAL = 1048576
X_DIM = 45
OUT_DIM = 10
ROWS = ROWS_TOTAL // N_CORES  # 131072 per core
P = 128                       # SBUF partitions
R = int(os.environ.get("K_R", "64"))          # rows per partition per tile
NTILES = ROWS // (P * R)
# Note: this neuronxcc/walrus build rejects TensorTensor on Pool
# ("Instruction engine check failed"), so the chain stays on DVE.
CHAIN_ENG = os.environ.get("K_CHAIN_ENG", "vector")
XT_BUFS = int(os.environ.get("K_XT_BUFS", "4"))
POOL_BUFS = int(os.environ.get("K_POOL_BUFS", "3"))
# Timing aid: repeat the whole tile sweep in-device so exec time dominates
# dispatch overhead. Output is idempotent across repeats. Default 1.
REPEAT = int(os.environ.get("K_REPEAT", "1"))
TRI = [g * (g - 1) // 2 for g in range(1, 11)]

_CACHE = {}


def _build_nc(r=None, repeat=None, xt_bufs=None, pool_bufs=None,
              chain_eng=None, dma_only=False):
    # Bacc (not bare Bass): finalize() runs generate_event_semaphores, which
    # splits multi-wait DMAs into event-semaphore + 1-wait DMA pairs.
    r = R if r is None else r
    repeat = REPEAT if repeat is None else repeat
    xt_bufs = XT_BUFS if xt_bufs is None else xt_bufs
    pool_bufs = POOL_BUFS if pool_bufs is None else pool_bufs
    chain_eng = CHAIN_ENG if chain_eng is None else chain_eng
    ntiles = ROWS // (P * r)
    nc = bacc.Bacc()
    x_d = nc.declare_dram_parameter("x", [ROWS, X_DIM], F32, isOutput=False)
    o_d = nc.declare_dram_parameter("o", [ROWS, OUT_DIM], F32, isOutput=True)

    x_v = x_d.rearrange("(n p r) d -> n p (r d)", p=P, r=r)
    o_v = o_d.rearrange("(n p r) e -> n p (r e)", p=P, r=r)

    with TileContext(nc) as tc:
        with tc.tile_pool(name="pool", bufs=pool_bufs) as pool:
            for n in [t for _ in range(repeat) for t in range(ntiles)]:
                xt = pool.tile([P, r * X_DIM], F32, tag="xt", bufs=xt_bufs)
                nc.sync.dma_start(xt[:], x_v[n])
                x3 = xt.rearrange("p (r d) -> p r d", d=X_DIM)
                R_ = r

                if dma_only:
                    # DMA-limited variant: out <- first 10 features of each
                    # row (no compute), to bound the pure-DMA sweep time.
                    # dma_only="in": input DMA only (output never written).
                    if dma_only != "in":
                        ot = pool.tile([P, R_ * OUT_DIM], F32, tag="ot")
                        o3d = ot.rearrange("p (r e) -> p r e", e=10)
                        nc.vector.tensor_copy(o3d, x3[:, :, 0:10])
                        nc.sync.dma_start(o_v[n], ot[:])
                    continue

                eng = nc.gpsimd if chain_eng == "gpsimd" else nc.vector

                # group maxes M_g (slot g-1), g=1..9
                Mst = pool.tile([P, R_ * 9], F32, tag="Mst")
                M3 = Mst.rearrange("p (r g) -> p r g", g=9)
                for g in range(1, 10):
                    nc.vector.tensor_reduce(
                        M3[:, :, g - 1], x3[:, :, TRI[g - 1]:TRI[g]],
                        axis=mybir.AxisListType.X, op=mybir.AluOpType.max,
                    )

                # offset-max chain, init folded: acc[0:8]=max(grp9[0:8],grp8)
                acc = pool.tile([P, R_ * 9], F32, tag="acc")
                a3 = acc.rearrange("p (r g) -> p r g", g=9)
                eng.tensor_tensor(
                    a3[:, :, 0:8], x3[:, :, TRI[8]:TRI[8] + 8],
                    x3[:, :, TRI[7]:TRI[8]], mybir.AluOpType.max,
                )
                eng.tensor_copy(a3[:, :, 8:9], x3[:, :, TRI[8] + 8:TRI[9]])
                for g in range(7, 0, -1):
                    eng.tensor_tensor(
                        a3[:, :, 0:g], a3[:, :, 0:g],
                        x3[:, :, TRI[g - 1]:TRI[g]], mybir.AluOpType.max,
                    )

                mrow = pool.tile([P, R_], F32, tag="mrow")
                nc.vector.tensor_reduce(
                    mrow[:], a3, axis=mybir.AxisListType.X,
                    op=mybir.AluOpType.max,
                )
                m_b9 = mrow.unsqueeze(2).broadcast_to([P, R_, 9])
                m_b1 = mrow.unsqueeze(2).broadcast_to([P, R_, 1])

                ot = pool.tile([P, R_ * OUT_DIM], F32, tag="ot")
                o3 = ot.rearrange("p (r e) -> p r e", e=10)

                # V-merge: slot k (k=1..8) feeds out col 9-k, needs group k
                # (M3 slot k-1): acc[1:9] = max(acc[1:9], M3[0:8]) in place
                nc.vector.tensor_tensor(
                    a3[:, :, 1:9], a3[:, :, 1:9], M3[:, :, 0:8],
                    mybir.AluOpType.max,
                )
                # cols 9..1 <- eq(acc[0:9], m) (reversed out AP)
                nc.vector.tensor_tensor(
                    o3[:, :, 1:10][:, :, ::-1], a3, m_b9,
                    mybir.AluOpType.is_equal,
                )
                # col 0 <- eq(M_9, m)
                nc.vector.tensor_tensor(
                    o3[:, :, 0:1], M3[:, :, 8:9], m_b1,
                    mybir.AluOpType.is_equal,
                )

                nc.sync.dma_start(o_v[n], ot[:])
    return nc


BF16 = mybir.dt.bfloat16
C = ROWS // P  # 1024 rows per partition in the transposed layout


def _build_nc_t(repeat=1, eq_mode="strided", gp_folds=0, cce_merge=False,
                cce_feats=(), gp_dma=0):
    """Transposed (feature-major) kernel: x arrives as [45, 131072] bf16.

    Each feature row f is one [128, 1024] SBUF tile (a single contiguous
    262KB DMA). All per-row work is then pure elementwise [128, 1024] tile
    ops on DVE (bf16 tensor_tensor runs in 2x perf mode):
      group maxes M_g   = TT-max fold over the g features of group g  (36)
      offset maxes A_r  = TT-max fold over features Tri(g-1)+r, g>r   (36)
      row max m         = TT-max fold over the 9 group maxes          (8)
      V-merge           = A_r <- max(A_r, M_g=r) for r=1..8           (8)
      one-hot eqs       = out col 0 <- eq(M_9,m); col c <- eq(A_{9-c},m)
                          c=1..8; col 9 <- eq(A_0,m)   (strided writes)
    Output tile [128, 1024*10] bf16 -> one 2.6MB DMA to o [131072, 10].
    bf16 rounding is monotone, so vs the f32 reference the argmax can only
    tie (never silently flip); any tie sets >=3 bits -> row-sum != 2 ->
    exact host fixup in kernel().
    """
    MAX = mybir.AluOpType.max
    EQ = mybir.AluOpType.is_equal
    nc = bacc.Bacc()
    x_d = nc.declare_dram_parameter("x", [X_DIM, ROWS], BF16, isOutput=False)
    o_d = nc.declare_dram_parameter("o", [ROWS, OUT_DIM], BF16, isOutput=True)
    x_v = x_d.rearrange("f (p c) -> f p c", p=P)
    o_v = o_d.rearrange("(p c) e -> p (c e)", p=P)

    cce = set(cce_feats)
    with TileContext(nc) as tc:
        with tc.tile_pool(name="pool", bufs=1) as pool:
            for _ in range(repeat):
                xf = {f: pool.tile([P, C], BF16, name=f"x{f}", tag=f"x{f}")
                      for f in range(X_DIM) if f not in cce}
                # high groups first: the offset folds start at g=9
                gpd = [gp_dma]
                for f in [f for g in range(9, 0, -1)
                          for f in range(TRI[g - 1], TRI[g])]:
                    if f not in cce:
                        if gpd[0] > 0:
                            gpd[0] -= 1
                            nc.gpsimd.dma_start(xf[f][:], x_v[f])
                        else:
                            nc.sync.dma_start(xf[f][:], x_v[f])

                # gp_folds > 0: route that many fold TTs through GpSimd to
                # split elementwise work across engines (compile check!).
                gp_left = [gp_folds]

                def tt_eng():
                    if gp_left[0] > 0:
                        gp_left[0] -= 1
                        return nc.gpsimd
                    return nc.vector

                def fold(tag, feats):
                    tts = [f for f in feats if f not in cce]
                    ccs = [f for f in feats if f in cce]
                    if len(feats) == 1:
                        return xf[feats[0]]
                    assert len(tts) >= 2, (tag, feats)
                    t = pool.tile([P, C], BF16, name=tag, tag=tag)
                    tt_eng().tensor_tensor(
                        t[:], xf[tts[0]][:], xf[tts[1]][:], MAX)
                    for f in tts[2:]:
                        tt_eng().tensor_tensor(t[:], t[:], xf[f][:], MAX)
                    # remaining members fold in via the DMA engines' inline
                    # CCE max (DRAM -> SBUF accumulate), off the DVE
                    for f in ccs:
                        nc.gpsimd.dma_start(t[:], x_v[f], accum_op=MAX)
                    return t

                Mg = {g: fold(f"M{g}", list(range(TRI[g - 1], TRI[g])))
                      for g in range(1, 10)}
                Ar = {r: fold(f"A{r}", [TRI[g - 1] + r for g in range(9, r, -1)])
                      for r in range(9)}

                m = pool.tile([P, C], BF16, tag="m")
                nc.vector.tensor_tensor(m[:], Mg[9][:], Mg[8][:], MAX)
                for g in range(7, 0, -1):
                    nc.vector.tensor_tensor(m[:], m[:], Mg[g][:], MAX)

                for r in range(1, 9):
                    if cce_merge:
                        # SBUF->SBUF accumulate-DMA merge (independent dests)
                        nc.gpsimd.dma_start(
                            Ar[r][:], Mg[r][:], accum_op=MAX)
                    else:
                        nc.vector.tensor_tensor(
                            Ar[r][:], Ar[r][:], Mg[r][:], MAX)

                ot = pool.tile([P, C * OUT_DIM], BF16, tag="ot")
                o3 = ot.rearrange("p (c e) -> p c e", e=OUT_DIM)
                m_u = m.unsqueeze(2)
                srcs = [Mg[9]] + [Ar[9 - c] for c in range(1, 9)] + [Ar[0]]
                if eq_mode == "strided":
                    # eq straight into the interleaved out tile (1x mode)
                    for c, s in enumerate(srcs):
                        nc.vector.tensor_tensor(
                            o3[:, :, c:c + 1], s.unsqueeze(2), m_u, EQ)
                else:
                    # eq into contiguous tiles (2x mode); ACT engine does the
                    # strided interleave copies in parallel with DVE
                    for c, s in enumerate(srcs):
                        e = pool.tile([P, C], BF16, name=f"E{c}", tag=f"E{c}")
                        nc.vector.tensor_tensor(e[:], s[:], m[:], EQ)
                        nc.scalar.copy(o3[:, :, c:c + 1], e.unsqueeze(2))
                nc.sync.dma_start(o_v, ot[:])
    return nc


# Shipping configuration for _build_nc_t (see bench_t.py for the A/B data).
VARIANT = dict(eq_mode="act")


def _get_nc():
    if "nc" not in _CACHE:
        nc = _build_nc_t(**VARIANT)
        if not nc.is_finalized():
            nc.finalize()  # Bacc: alloc_regs + generate_event_semaphores
        _CACHE["nc"] = nc
    return _CACHE["nc"]


def _to_bf16(x):
    """Vectorized round-to-nearest-even f32 -> bf16 (monotone, matches HW)."""
    import ml_dtypes

    u = np.ascontiguousarray(x).view(np.uint32)
    r = (u + 0x7FFF + ((u >> 16) & 1)) >> 16
    return r.astype(np.uint16).view(ml_dtypes.bfloat16)


def make_in_maps(x):
    """Host prep: bf16 downcast + per-core feature-major transpose."""
    xb = _to_bf16(np.asarray(x, np.float32))
    return [
        {"x": np.ascontiguousarray(xb[c * ROWS:(c + 1) * ROWS].T)}
        for c in range(N_CORES)
    ]


def run_on_hw(x, trace=False, **kw):
    """Run the SPMD kernel on the 8 cores; returns (out, results)."""
    nc = _get_nc()
    res = run_bass_kernel_spmd(
        nc, make_in_maps(x), list(range(N_CORES)), trace=trace, **kw
    )
    out = np.concatenate(
        [np.asarray(r["o"]).astype(np.float32) for r in res.results], axis=0
    )
    return out, res


def kernel(x, twohot_table):
    x = np.asarray(x, dtype=np.float32)
    table = np.asarray(twohot_table, dtype=np.float32)
    assert x.shape == (ROWS_TOTAL, X_DIM), x.shape

    out, _ = run_on_hw(x)

    # Tie fixup: a tie at the (bf16) max sets >=3 bits on that row
    # (row-sum != 2); bf16 rounding is monotone so non-tied rows match the
    # f32 argmax exactly. Recompute tied rows exactly on host from f32 x.
    bad = out.sum(axis=1) != 2.0
    if bad.any():
        out[bad] = table[x[bad].argmax(axis=1)]
    return out



# revision 19
# speedup vs baseline: 98.7973x; 1.9029x over previous
"""Trainium2 Bass kernel for nn_CompressSensory (embedding_lookup):
out = twohot_table[argmax(x, axis=1)] for x [1048576, 45] f32.

Strategy: pure data parallel over 8 NeuronCores (131072 rows each), in a
TRANSPOSED feature-major layout. The host ships x as bf16 [45, 131072] per
core; each feature row is one [128, 1024] SBUF tile (a single contiguous
262KB DMA, 45 total). All per-row work is then pure elementwise [128,1024]
tile ops on the DVE, where bf16 tensor_tensor runs in 2x perf mode:
  group maxes M_g  = TT-max fold over the g features of group g     (36 TT)
  offset maxes A_r = TT-max fold over features Tri(g-1)+r, g>r      (36 TT)
  row max m        = TT-max fold over the 9 group maxes              (8 TT)
  V-merge          = A_r <- max(A_r, M_r) for r=1..8                 (8 TT)
  one-hots         = eq(M_9,m) -> col 0; eq(A_{9-c},m) -> col c      (10 TT)
Features are loaded as 9 contiguous ROW tiles (row g = the g features of
group g side by side, one DMA each), which turns the offset folds into a
shifted chain of 8 WIDE TTs (8k..1k elems) and the merge into one wide 8k
TT; the chain is interleaved with the group folds so each row tile frees
right after its last reader, big rows first -- the order the next sweep's
chain consumes them (cross-sweep overlap, acc/mg/m double-buffered). The
eq one-hots recycle the dead mg slices (2x mode); the ACT engine
interleaves the 10 columns into the output tile in parallel with DVE; one
2.6MB DMA stores o [131072, 10] bf16.

Correctness: bf16 rounding is monotone, so vs the f32 reference the argmax
can only TIE (never silently flip); any tie sets >=3 bits -> row-sum != 2
-> those rows (~1.2%) are recomputed exactly on host from f32 x.
"""

import os

import numpy as np

# Whole-tile dep granularity keeps per-instruction sync-wait counts low
# (walrus rejects DMA pseudo-instructions with >1 sync wait).
os.environ.setdefault("BY_DEFAULT_DISABLE_SUBTILE_DEPS", "1")

import concourse.bass as bass
import concourse.bacc as bacc
import concourse.mybir as mybir
from concourse.tile import TileContext
from concourse.bass_utils import run_bass_kernel_spmd

F32 = mybir.dt.float32
N_CORES = 8
ROWS_TOTAL = 1048576
X_DIM = 45
OUT_DIM = 10
ROWS = ROWS_TOTAL // N_CORES  # 131072 per core
P = 128                       # SBUF partitions
R = int(os.environ.get("K_R", "64"))          # rows per partition per tile
NTILES = ROWS // (P * R)
# Note: this neuronxcc/walrus build rejects TensorTensor on Pool
# ("Instruction engine check failed"), so the chain stays on DVE.
CHAIN_ENG = os.environ.get("K_CHAIN_ENG", "vector")
XT_BUFS = int(os.environ.get("K_XT_BUFS", "4"))
POOL_BUFS = int(os.environ.get("K_POOL_BUFS", "3"))
# Timing aid: repeat the whole tile sweep in-device so exec time dominates
# dispatch overhead. Output is idempotent across repeats. Default 1.
REPEAT = int(os.environ.get("K_REPEAT", "1"))
TRI = [g * (g - 1) // 2 for g in range(1, 11)]

_CACHE = {}


def _build_nc(r=None, repeat=None, xt_bufs=None, pool_bufs=None,
              chain_eng=None, dma_only=False):
    # Bacc (not bare Bass): finalize() runs generate_event_semaphores, which
    # splits multi-wait DMAs into event-semaphore + 1-wait DMA pairs.
    r = R if r is None else r
    repeat = REPEAT if repeat is None else repeat
    xt_bufs = XT_BUFS if xt_bufs is None else xt_bufs
    pool_bufs = POOL_BUFS if pool_bufs is None else pool_bufs
    chain_eng = CHAIN_ENG if chain_eng is None else chain_eng
    ntiles = ROWS // (P * r)
    nc = bacc.Bacc()
    x_d = nc.declare_dram_parameter("x", [ROWS, X_DIM], F32, isOutput=False)
    o_d = nc.declare_dram_parameter("o", [ROWS, OUT_DIM], F32, isOutput=True)

    x_v = x_d.rearrange("(n p r) d -> n p (r d)", p=P, r=r)
    o_v = o_d.rearrange("(n p r) e -> n p (r e)", p=P, r=r)

    with TileContext(nc) as tc:
        with tc.tile_pool(name="pool", bufs=pool_bufs) as pool:
            for n in [t for _ in range(repeat) for t in range(ntiles)]:
                xt = pool.tile([P, r * X_DIM], F32, tag="xt", bufs=xt_bufs)
                nc.sync.dma_start(xt[:], x_v[n])
                x3 = xt.rearrange("p (r d) -> p r d", d=X_DIM)
                R_ = r

                if dma_only:
                    # DMA-limited variant: out <- first 10 features of each
                    # row (no compute), to bound the pure-DMA sweep time.
                    # dma_only="in": input DMA only (output never written).
                    if dma_only != "in":
                        ot = pool.tile([P, R_ * OUT_DIM], F32, tag="ot")
                        o3d = ot.rearrange("p (r e) -> p r e", e=10)
                        nc.vector.tensor_copy(o3d, x3[:, :, 0:10])
                        nc.sync.dma_start(o_v[n], ot[:])
                    continue

                eng = nc.gpsimd if chain_eng == "gpsimd" else nc.vector

                # group maxes M_g (slot g-1), g=1..9
                Mst = pool.tile([P, R_ * 9], F32, tag="Mst")
                M3 = Mst.rearrange("p (r g) -> p r g", g=9)
                for g in range(1, 10):
                    nc.vector.tensor_reduce(
                        M3[:, :, g - 1], x3[:, :, TRI[g - 1]:TRI[g]],
                        axis=mybir.AxisListType.X, op=mybir.AluOpType.max,
                    )

                # offset-max chain, init folded: acc[0:8]=max(grp9[0:8],grp8)
                acc = pool.tile([P, R_ * 9], F32, tag="acc")
                a3 = acc.rearrange("p (r g) -> p r g", g=9)
                eng.tensor_tensor(
                    a3[:, :, 0:8], x3[:, :, TRI[8]:TRI[8] + 8],
                    x3[:, :, TRI[7]:TRI[8]], mybir.AluOpType.max,
                )
                eng.tensor_copy(a3[:, :, 8:9], x3[:, :, TRI[8] + 8:TRI[9]])
                for g in range(7, 0, -1):
                    eng.tensor_tensor(
                        a3[:, :, 0:g], a3[:, :, 0:g],
                        x3[:, :, TRI[g - 1]:TRI[g]], mybir.AluOpType.max,
                    )

                mrow = pool.tile([P, R_], F32, tag="mrow")
                nc.vector.tensor_reduce(
                    mrow[:], a3, axis=mybir.AxisListType.X,
                    op=mybir.AluOpType.max,
                )
                m_b9 = mrow.unsqueeze(2).broadcast_to([P, R_, 9])
                m_b1 = mrow.unsqueeze(2).broadcast_to([P, R_, 1])

                ot = pool.tile([P, R_ * OUT_DIM], F32, tag="ot")
                o3 = ot.rearrange("p (r e) -> p r e", e=10)

                # V-merge: slot k (k=1..8) feeds out col 9-k, needs group k
                # (M3 slot k-1): acc[1:9] = max(acc[1:9], M3[0:8]) in place
                nc.vector.tensor_tensor(
                    a3[:, :, 1:9], a3[:, :, 1:9], M3[:, :, 0:8],
                    mybir.AluOpType.max,
                )
                # cols 9..1 <- eq(acc[0:9], m) (reversed out AP)
                nc.vector.tensor_tensor(
                    o3[:, :, 1:10][:, :, ::-1], a3, m_b9,
                    mybir.AluOpType.is_equal,
                )
                # col 0 <- eq(M_9, m)
                nc.vector.tensor_tensor(
                    o3[:, :, 0:1], M3[:, :, 8:9], m_b1,
                    mybir.AluOpType.is_equal,
                )

                nc.sync.dma_start(o_v[n], ot[:])
    return nc


BF16 = mybir.dt.bfloat16
C = ROWS // P  # 1024 rows per partition in the transposed layout


def _build_nc_t(repeat=1, eq_mode="strided", gp_folds=0, cce_merge=False,
                cce_feats=(), gp_dma=0, layout="tiles"):
    if layout == "rows":
        return _build_nc_w(repeat=repeat)
    """Transposed (feature-major) kernel: x arrives as [45, 131072] bf16.

    Each feature row f is one [128, 1024] SBUF tile (a single contiguous
    262KB DMA). All per-row work is then pure elementwise [128, 1024] tile
    ops on DVE (bf16 tensor_tensor runs in 2x perf mode):
      group maxes M_g   = TT-max fold over the g features of group g  (36)
      offset maxes A_r  = TT-max fold over features Tri(g-1)+r, g>r   (36)
      row max m         = TT-max fold over the 9 group maxes          (8)
      V-merge           = A_r <- max(A_r, M_g=r) for r=1..8           (8)
      one-hot eqs       = out col 0 <- eq(M_9,m); col c <- eq(A_{9-c},m)
                          c=1..8; col 9 <- eq(A_0,m)   (strided writes)
    Output tile [128, 1024*10] bf16 -> one 2.6MB DMA to o [131072, 10].
    bf16 rounding is monotone, so vs the f32 reference the argmax can only
    tie (never silently flip); any tie sets >=3 bits -> row-sum != 2 ->
    exact host fixup in kernel().
    """
    MAX = mybir.AluOpType.max
    EQ = mybir.AluOpType.is_equal
    nc = bacc.Bacc()
    x_d = nc.declare_dram_parameter("x", [X_DIM, ROWS], BF16, isOutput=False)
    o_d = nc.declare_dram_parameter("o", [ROWS, OUT_DIM], BF16, isOutput=True)
    x_v = x_d.rearrange("f (p c) -> f p c", p=P)
    o_v = o_d.rearrange("(p c) e -> p (c e)", p=P)

    cce = set(cce_feats)
    with TileContext(nc) as tc:
        with tc.tile_pool(name="pool", bufs=1) as pool:
            for _ in range(repeat):
                xf = {f: pool.tile([P, C], BF16, name=f"x{f}", tag=f"x{f}")
                      for f in range(X_DIM) if f not in cce}
                # high groups first: the offset folds start at g=9
                gpd = [gp_dma]
                for f in [f for g in range(9, 0, -1)
                          for f in range(TRI[g - 1], TRI[g])]:
                    if f not in cce:
                        if gpd[0] > 0:
                            gpd[0] -= 1
                            nc.gpsimd.dma_start(xf[f][:], x_v[f])
                        else:
                            nc.sync.dma_start(xf[f][:], x_v[f])

                # gp_folds > 0: route that many fold TTs through GpSimd to
                # split elementwise work across engines (compile check!).
                gp_left = [gp_folds]

                def tt_eng():
                    if gp_left[0] > 0:
                        gp_left[0] -= 1
                        return nc.gpsimd
                    return nc.vector

                def fold(tag, feats):
                    tts = [f for f in feats if f not in cce]
                    ccs = [f for f in feats if f in cce]
                    if len(feats) == 1:
                        return xf[feats[0]]
                    assert len(tts) >= 2, (tag, feats)
                    t = pool.tile([P, C], BF16, name=tag, tag=tag)
                    tt_eng().tensor_tensor(
                        t[:], xf[tts[0]][:], xf[tts[1]][:], MAX)
                    for f in tts[2:]:
                        tt_eng().tensor_tensor(t[:], t[:], xf[f][:], MAX)
                    # remaining members fold in via the DMA engines' inline
                    # CCE max (DRAM -> SBUF accumulate), off the DVE
                    for f in ccs:
                        nc.gpsimd.dma_start(t[:], x_v[f], accum_op=MAX)
                    return t

                Mg = {g: fold(f"M{g}", list(range(TRI[g - 1], TRI[g])))
                      for g in range(1, 10)}
                Ar = {r: fold(f"A{r}", [TRI[g - 1] + r for g in range(9, r, -1)])
                      for r in range(9)}

                m = pool.tile([P, C], BF16, tag="m")
                nc.vector.tensor_tensor(m[:], Mg[9][:], Mg[8][:], MAX)
                for g in range(7, 0, -1):
                    nc.vector.tensor_tensor(m[:], m[:], Mg[g][:], MAX)

                for r in range(1, 9):
                    if cce_merge:
                        # SBUF->SBUF accumulate-DMA merge (independent dests)
                        nc.gpsimd.dma_start(
                            Ar[r][:], Mg[r][:], accum_op=MAX)
                    else:
                        nc.vector.tensor_tensor(
                            Ar[r][:], Ar[r][:], Mg[r][:], MAX)

                ot = pool.tile([P, C * OUT_DIM], BF16, tag="ot")
                o3 = ot.rearrange("p (c e) -> p c e", e=OUT_DIM)
                m_u = m.unsqueeze(2)
                srcs = [Mg[9]] + [Ar[9 - c] for c in range(1, 9)] + [Ar[0]]
                if eq_mode == "strided":
                    # eq straight into the interleaved out tile (1x mode)
                    for c, s in enumerate(srcs):
                        nc.vector.tensor_tensor(
                            o3[:, :, c:c + 1], s.unsqueeze(2), m_u, EQ)
                else:
                    # eq into contiguous tiles (2x mode); ACT engine does the
                    # strided interleave copies in parallel with DVE
                    for c, s in enumerate(srcs):
                        e = pool.tile([P, C], BF16, name=f"E{c}", tag=f"E{c}")
                        nc.vector.tensor_tensor(e[:], s[:], m[:], EQ)
                        nc.scalar.copy(o3[:, :, c:c + 1], e.unsqueeze(2))
                nc.sync.dma_start(o_v, ot[:])
    return nc


def _build_nc_w(repeat=1):
    """Wide-instruction variant of the transposed kernel.

    Features load as 9 contiguous ROW tiles (row g = the g features of
    group g side by side, one DMA each; the host layout is unchanged since
    group features are adjacent in xT). Offsets align across rows, so:
      offset maxes = the shifted chain as 8 WIDE TTs (8k..1k elems) + copy
      group maxes  = left-folds within each row tile -> mg slices  (36 TT)
      merge        = ONE wide 8k TT: acc[1:9] <- max(acc[1:9], mg[0:8])
      row max m    = left-fold over mg slices                       (8 TT)
      eq           = col0 -> et; cols 1..9 write into the dead mg slices
                     (mg is fully consumed once col0's eq has read M_9)
    acc/mg/m are double-buffered so the next sweep's folds overlap this
    sweep's eq/store tail. ACT interleaves the 10 one-hot columns into ot.
    """
    MAX = mybir.AluOpType.max
    EQ = mybir.AluOpType.is_equal
    nc = bacc.Bacc()
    x_d = nc.declare_dram_parameter("x", [X_DIM, ROWS], BF16, isOutput=False)
    o_d = nc.declare_dram_parameter("o", [ROWS, OUT_DIM], BF16, isOutput=True)
    # [128, 45, 1024]: feature j's columns for partition p at [p, j, :]
    x_p = x_d.rearrange("f (p c) -> p f c", p=P)
    o_v = o_d.rearrange("(p c) e -> p (c e)", p=P)

    with TileContext(nc) as tc:
        with tc.tile_pool(name="pool", bufs=1) as pool:
            for _ in range(repeat):
                rt = {}
                for g in range(9, 0, -1):
                    t = pool.tile([P, g * C], BF16, name=f"row{g}",
                                  tag=f"row{g}")
                    rt[g] = t
                    t3 = t.rearrange("p (j c) -> p j c", c=C)
                    nc.sync.dma_start(t3, x_p[:, TRI[g - 1]:TRI[g]])

                # offset chain (wide TTs) interleaved with the group folds
                # so each row tile is freed right after its LAST reader —
                # big rows first, which is the order the NEXT sweep's chain
                # consumes them (avoids a cross-sweep DMA stall).
                acc = pool.tile([P, 9 * C], BF16, tag="acc", bufs=2)
                mg = pool.tile([P, 9 * C], BF16, tag="mg", bufs=2)

                def gfold(g):
                    # mg slice g-1 <- max over row g
                    d = mg[:, (g - 1) * C:g * C]
                    if g == 1:
                        nc.vector.tensor_copy(d, rt[1][:])
                        return
                    nc.vector.tensor_tensor(
                        d, rt[g][:, 0:C], rt[g][:, C:2 * C], MAX)
                    for j in range(2, g):
                        nc.vector.tensor_tensor(
                            d, d, rt[g][:, j * C:(j + 1) * C], MAX)

                nc.vector.tensor_tensor(
                    acc[:, 0:8 * C], rt[9][:, 0:8 * C], rt[8][:], MAX)
                nc.vector.tensor_copy(acc[:, 8 * C:9 * C], rt[9][:, 8 * C:])
                gfold(9)
                gfold(8)
                for g in range(7, 0, -1):
                    nc.vector.tensor_tensor(
                        acc[:, 0:g * C], acc[:, 0:g * C], rt[g][:], MAX)
                    gfold(g)

                # merge (one wide TT), then row max m
                nc.vector.tensor_tensor(
                    acc[:, C:9 * C], acc[:, C:9 * C], mg[:, 0:8 * C], MAX)
                m = pool.tile([P, C], BF16, tag="m", bufs=2)
                nc.vector.tensor_tensor(m[:], mg[:, 8 * C:], mg[:, 7 * C:8 * C],
                                        MAX)
                for g in range(7, 0, -1):
                    nc.vector.tensor_tensor(
                        m[:], m[:], mg[:, (g - 1) * C:g * C], MAX)

                # one-hots: col0 from M_9 (before mg is recycled), then the
                # dead mg slices take cols 1..9
                ot = pool.tile([P, C * OUT_DIM], BF16, tag="ot")
                o3 = ot.rearrange("p (c e) -> p c e", e=OUT_DIM)
                et = pool.tile([P, C], BF16, tag="et", bufs=2)
                nc.vector.tensor_tensor(et[:], mg[:, 8 * C:], m[:], EQ)
                nc.scalar.copy(o3[:, :, 0:1], et.unsqueeze(2))
                for c in range(1, 10):
                    dst = mg[:, (c - 1) * C:c * C]
                    nc.vector.tensor_tensor(
                        dst, acc[:, (9 - c) * C:(10 - c) * C], m[:], EQ)
                    nc.scalar.copy(o3[:, :, c:c + 1], dst.unsqueeze(2))
                nc.sync.dma_start(o_v, ot[:])
    return nc


# Shipping configuration (see bench_t.py for the A/B data).
VARIANT = dict(layout="rows")


def _get_nc():
    if "nc" not in _CACHE:
        nc = _build_nc_t(**VARIANT)
        if not nc.is_finalized():
            nc.finalize()  # Bacc: alloc_regs + generate_event_semaphores
        _CACHE["nc"] = nc
    return _CACHE["nc"]


def _to_bf16(x):
    """Vectorized round-to-nearest-even f32 -> bf16 (monotone, matches HW)."""
    import ml_dtypes

    u = np.ascontiguousarray(x).view(np.uint32)
    r = (u + 0x7FFF + ((u >> 16) & 1)) >> 16
    return r.astype(np.uint16).view(ml_dtypes.bfloat16)


def make_in_maps(x):
    """Host prep: bf16 downcast + per-core feature-major transpose."""
    xb = _to_bf16(np.asarray(x, np.float32))
    return [
        {"x": np.ascontiguousarray(xb[c * ROWS:(c + 1) * ROWS].T)}
        for c in range(N_CORES)
    ]


def run_on_hw(x, trace=False, **kw):
    """Run the SPMD kernel on the 8 cores; returns (out, results)."""
    nc = _get_nc()
    res = run_bass_kernel_spmd(
        nc, make_in_maps(x), list(range(N_CORES)), trace=trace, **kw
    )
    out = np.concatenate(
        [np.asarray(r["o"]).astype(np.float32) for r in res.results], axis=0
    )
    return out, res


def kernel(x, twohot_table):
    x = np.asarray(x, dtype=np.float32)
    table = np.asarray(twohot_table, dtype=np.float32)
    assert x.shape == (ROWS_TOTAL, X_DIM), x.shape

    out, _ = run_on_hw(x)

    # Tie fixup: a tie at the (bf16) max sets >=3 bits on that row
    # (row-sum != 2); bf16 rounding is monotone so non-tied rows match the
    # f32 argmax exactly. Recompute tied rows exactly on host from f32 x.
    bad = out.sum(axis=1) != 2.0
    if bad.any():
        out[bad] = table[x[bad].argmax(axis=1)]
    return out

